# revision 1
# baseline (speedup 1.0000x reference)
"""Two-layer GAT on 8 Trainium2 NeuronCores (Bass/Tile SPMD kernel).

Full inputs in, full output out. Internally:
  - host: bin-pack nodes into (core, tile, row) slots, build per-core edge
    index metadata (int16 gather indices, per-tile local dst ids).
  - device (SPMD, 8 cores):
    Phase A: per-shard table1 rows [h1 interleaved w/ ones | al_s | al_d | pad]
             (320 f32) via PE matmuls from host-pre-transposed x.
    AllGather table1 -> every core has all 50176 rows.
    Phase B: per dst-tile: dma_gather source rows, one-hot segment-softmax
             (exp without max subtraction; logits are bounded), PSUM-
             accumulated segment sums, ELU(+1) fused, layer-2 table rows
             [h2 | 1 | al_s2 | al_d2 | pad] (128 f32) built per tile.
    AllGather table2.
    Phase D: same edge machinery for layer 2 -> output rows.
"""

import heapq
import numpy as np

import concourse.bacc as bacc
import concourse.bass as bass
import concourse.mybir as mybir
import concourse.tile as tile
from concourse.bass_utils import run_bass_kernel_spmd

dt = mybir.dt
f32 = dt.float32
NEG_SLOPE = 0.2
CLAMP = 60.0


class Cfg:
    def __init__(self, n, f_in=128, heads=4, hid=64, out_ch=64,
                 ncores=8, nt=49, loch=12, hich=7, split=32768,
                 seg_f32r=False, h_f32r=False):
        self.n = n                    # real node count
        self.f_in = f_in
        self.heads = heads
        self.hid = hid
        self.out_ch = out_ch
        self.ncores = ncores
        self.nt = nt                  # dst tiles per core
        self.R = 128
        self.ntr = nt * 128           # rows per core
        self.slots = ncores * self.ntr
        self.loch = loch              # lo-src gather chunks per tile
        self.hich = hich
        self.cpt = loch + hich        # chunks per tile
        self.split = split            # table row split for int16 indexing
        self.seg_f32r = seg_f32r
        self.h_f32r = h_f32r
        self.t1w = 320                # table1 row width (f32), mult of 64
        self.t2w = 128                # table2 row width
        self.v1 = heads * (hid + 1)   # 260: [h|1] x heads
        self.v2 = out_ch + 1          # 65
        assert n <= self.slots
        assert self.split <= 32768
        assert self.slots - self.split <= 32768
        assert heads * hid + heads * 2 + heads <= self.t1w
        assert out_ch + 3 <= self.t2w


FULL = Cfg(n=50000)


# ---------------------------------------------------------------------------
# Host-side preprocessing
# ---------------------------------------------------------------------------

def pack_nodes(cfg, dst_nodes):
    """Assign each node to a (tile, row) slot, balancing edge counts per tile.

    dst_nodes: int array of edge destinations (self loops included).
    Returns perm[n] -> global slot id.
    """
    n, ntile = cfg.n, cfg.ncores * cfg.nt
    deg = np.bincount(dst_nodes, minlength=n)
    order = np.argsort(-deg, kind="stable")
    cap_edges = cfg.cpt * 128
    # greedy: highest degree first into least-loaded tile with a free row
    heap = [(0, t) for t in range(ntile)]
    heapq.heapify(heap)
    rows_used = np.zeros(ntile, np.int32)
    load = np.zeros(ntile, np.int64)
    tile_of = np.empty(n, np.int32)
    row_of = np.empty(n, np.int32)
    spill = []
    for nd in order:
        d = int(deg[nd])
        while True:
            l, t = heapq.heappop(heap)
            if rows_used[t] >= 128:
                continue  # tile full, drop from heap
            if l + d > cap_edges and l > 0:
                # shouldn't happen with sane caps; put node in next tile
                spill.append((l, t))
                continue
            break
        for item in spill:
            heapq.heappush(heap, item)
        spill.clear()
        tile_of[nd] = t
        row_of[nd] = rows_used[t]
        rows_used[t] += 1
        load[t] += d
        heapq.heappush(heap, (l + d, t))
    perm = tile_of.astype(np.int64) * 128 + row_of
    return perm


def prep_host(cfg, x, edge_index, W1, a_src1, a_dst1, b1, W2, a_src2, a_dst2, b2):
    n = cfg.n
    heads, hid, out_ch = cfg.heads, cfg.hid, cfg.out_ch
    x = np.asarray(x, np.float32)
    ei = np.asarray(edge_index, np.int64)
    # self loops
    loops = np.arange(n, dtype=np.int64)
    src = np.concatenate([ei[0], loops])
    dst = np.concatenate([ei[1], loops])

    perm = pack_nodes(cfg, dst)               # node -> slot
    s_src = perm[src]
    s_dst = perm[dst]
    tile_g = s_dst // 128                      # global tile id
    r_dst = s_dst % 128
    is_lo = s_src < cfg.split

    nt_all = cfg.ncores * cfg.nt
    lo_slots = cfg.loch * 128
    hi_slots = cfg.hich * 128

    idx_lo = np.zeros((nt_all, lo_slots), np.int16)
    idx_hi = np.zeros((nt_all, hi_slots), np.int16)
    dst_loc = np.full((nt_all, cfg.cpt * 128), -1.0, np.float32)

    # group edges by (tile, lo/hi)
    key = tile_g * 2 + (~is_lo).astype(np.int64)
    order = np.argsort(key, kind="stable")
    ks = key[order]
    bounds = np.searchsorted(ks, np.arange(2 * nt_all + 1))
    for t in range(nt_all):
        elo = order[bounds[2 * t]:bounds[2 * t + 1]]
        ehi = order[bounds[2 * t + 1]:bounds[2 * t + 2]]
        nlo, nhi = len(elo), len(ehi)
        if nlo > lo_slots or nhi > hi_slots:
            raise RuntimeError(f"tile {t} overflow: lo={nlo} hi={nhi}")
        idx_lo[t, :nlo] = s_src[elo].astype(np.int16)
        idx_hi[t, :nhi] = (s_src[ehi] - cfg.split).astype(np.int16)
        dst_loc[t, :nlo] = r_dst[elo]
        dst_loc[t, lo_slots:lo_slots + nhi] = r_dst[ehi]

    def wrap16(a):
        # [nt, slots] -> [nt, 16, slots/16] wrapped -> concat -> [16, nt*slots/16]
        ntl, s = a.shape
        w = a.reshape(ntl, s // 16, 16).transpose(0, 2, 1)   # [nt,16,s/16]
        w = w.transpose(1, 0, 2).reshape(16, ntl * (s // 16))
        return np.tile(w, (8, 1)).copy()                     # replicate to 128 parts

    # per-core slices
    npc = cfg.nt  # tiles per core
    in_maps = []
    # weight folding (host, params only)
    Asrc = np.zeros((heads * hid, heads), np.float32)
    Adst = np.zeros((heads * hid, heads), np.float32)
    for h in range(heads):
        Asrc[h * hid:(h + 1) * hid, h] = a_src1[h]
        Adst[h * hid:(h + 1) * hid, h] = a_dst1[h]
    W1aug = np.zeros((cfg.f_in, heads * (hid + 1) + 2 * heads), np.float32)
    for h in range(heads):
        W1aug[:, h * (hid + 1):h * (hid + 1) + hid] = W1[:, h * hid:(h + 1) * hid]
    W1aug[:, cfg.v1:cfg.v1 + heads] = W1 @ Asrc
    W1aug[:, cfg.v1 + heads:cfg.v1 + 2 * heads] = W1 @ Adst

    W2aug = np.concatenate([W2, (W2 @ a_src2[0])[:, None],
                            (W2 @ a_dst2[0])[:, None]], axis=1).astype(np.float32)
    caug = (-W2aug.sum(axis=0, keepdims=True)).astype(np.float32)  # [1, 66]

    assert not np.any(np.asarray(b1)), "nonzero b1 unsupported in this build"
    assert not np.any(np.asarray(b2)), "nonzero b2 unsupported in this build"

    i128 = np.eye(128, dtype=np.float32)
    iotarow = np.tile(np.arange(128, dtype=np.float32), (128, 1)).copy()

    # xT per core: [f_in, ntr], dummies zero
    inv = np.zeros(cfg.slots, np.int64) - 1
    inv[perm] = np.arange(n)
    xs = np.zeros((cfg.slots, cfg.f_in), np.float32)
    xs[perm] = x

    idx_lo_w = wrap16(idx_lo)   # [128, nt_all*lo_slots/16]
    idx_hi_w = wrap16(idx_hi)
    lo_cols = lo_slots // 16
    hi_cols = hi_slots // 16

    for c in range(cfg.ncores):
        t0, t1 = c * npc, (c + 1) * npc
        m = {
            "xT": xs[c * cfg.ntr:(c + 1) * cfg.ntr].T.copy(),
            "w1aug": W1aug,
            "w2aug": np.stack([W2aug[:128], W2aug[128:]]) if W2aug.shape[0] == 256
                     else np.stack([W2aug, np.zeros_like(W2aug)]),
            "caug": caug,
            "i128": i128,
            "iotarow": iotarow,
            "idxlo": idx_lo_w[:, t0 * lo_cols:t1 * lo_cols].copy(),
            "idxhi": idx_hi_w[:, t0 * hi_cols:t1 * hi_cols].copy(),
            "dstloc": dst_loc[t0:t1].reshape(npc, cfg.cpt, 128)
                      .transpose(2, 0, 1).reshape(128, npc * cfg.cpt).copy(),
        }
        in_maps.append(m)
    return in_maps, perm


# ---------------------------------------------------------------------------
# Device program
# ---------------------------------------------------------------------------

def build_program(cfg):
    H, HID = cfg.heads, cfg.hid
    V1, V2 = cfg.v1, cfg.v2
    T1W, T2W = cfg.t1w, cfg.t2w
    NT, CPT, LOCH, HICH = cfg.nt, cfg.cpt, cfg.loch, cfg.hich
    NTR = cfg.ntr
    OUT = cfg.out_ch
    K2 = H * HID  # layer2 input dim (256)
    n_k2 = (K2 + 127) // 128

    nc = bacc.Bacc("TRN2", target_bir_lowering=False, debug=False,
                   num_devices=cfg.ncores)

    # I/O
    xT = nc.dram_tensor("xT", [cfg.f_in, NTR], f32, kind="ExternalInput")
    w1aug_d = nc.dram_tensor("w1aug", [cfg.f_in, V1 + 2 * H], f32, kind="ExternalInput")
    w2aug_d = nc.dram_tensor("w2aug", [2, 128, V2 + 1], f32, kind="ExternalInput")
    caug_d = nc.dram_tensor("caug", [1, V2 + 1], f32, kind="ExternalInput")
    i128_d = nc.dram_tensor("i128", [128, 128], f32, kind="ExternalInput")
    iota_d = nc.dram_tensor("iotarow", [128, 128], f32, kind="ExternalInput")
    idxlo_d = nc.dram_tensor("idxlo", [128, NT * LOCH * 8], dt.int16, kind="ExternalInput")
    idxhi_d = nc.dram_tensor("idxhi", [128, NT * HICH * 8], dt.int16, kind="ExternalInput")
    dstloc_d = nc.dram_tensor("dstloc", [128, NT * CPT], f32, kind="ExternalInput")
    out_d = nc.dram_tensor("out_shard", [NTR, OUT], f32, kind="ExternalOutput")

    tbl1_shard = nc.dram_tensor("tbl1_shard", [NTR, T1W], f32)
    tbl1 = nc.dram_tensor("tbl1", [cfg.slots, T1W], f32, addr_space="Shared")
    tbl2_shard = nc.dram_tensor("tbl2_shard", [NTR, T2W], f32)
    tbl2 = nc.dram_tensor("tbl2", [cfg.slots, T2W], f32, addr_space="Shared")

    rg = [list(range(cfg.ncores))]

    mm_dt = dt.float32r if cfg.h_f32r else f32
    seg_dt = dt.float32r if cfg.seg_f32r else f32

    def c(ap, d):
        return ap.bitcast(d) if d != f32 else ap

    with tile.TileContext(nc) as tc:
        with tc.tile_pool(name="res", bufs=1) as res:
            w1a = res.tile([cfg.f_in, V1 + 2 * H], f32)
            w2a = res.tile([128, 2 * (V2 + 1)], f32)
            ca = res.tile([1, V2 + 1], f32)
            i128 = res.tile([128, 128], f32)
            iota = res.tile([128, 128], f32)
            idxlo = res.tile([128, NT * LOCH * 8], dt.int16)
            idxhi = res.tile([128, NT * HICH * 8], dt.int16)
            dstloc = res.tile([128, NT * CPT], f32)
            alds = res.tile([128, NT * H], f32)
            ald2 = res.tile([128, NT], f32)
            ones_row = res.tile([1, 128], f32)

            nc.sync.dma_start(w1a[:], w1aug_d[:])
            for j in range(2):
                nc.sync.dma_start(w2a[:, j * (V2 + 1):(j + 1) * (V2 + 1)],
                                  w2aug_d[j, :, :])
            nc.sync.dma_start(ca[:], caug_d[:])
            nc.sync.dma_start(i128[:], i128_d[:])
            nc.sync.dma_start(iota[:], iota_d[:])
            nc.sync.dma_start(idxlo[:], idxlo_d[:])
            nc.sync.dma_start(idxhi[:], idxhi_d[:])
            nc.sync.dma_start(dstloc[:], dstloc_d[:])
            nc.gpsimd.memset(ones_row[:], 1.0)

            # ------------------------------------------------------------
            # Phase A: table1 shard rows = [W1aug' cols of x | pad]
            # ------------------------------------------------------------
            w1cols = V1 + 2 * H  # 268
            nrem = w1cols - 256  # 12
            with tc.tile_pool(name="pa_sb", bufs=2) as pa, \
                 tc.tile_pool(name="pa_ps", bufs=2, space="PSUM") as pap:
                for t in range(NT):
                    xg = pa.tile([128, 128], f32, tag="xg")
                    nc.sync.dma_start(xg[:], xT[:, t * 128:(t + 1) * 128])
                    h1t_ps = pap.tile([128, 128], f32, tag="h1a")
                    h1t_ps2 = pap.tile([128, 128], f32, tag="h1b")
                    h1t_ps3 = pap.tile([64, 128], f32, tag="h1c")
                    nc.tensor.matmul(h1t_ps[:], c(w1a[:, 0:128], mm_dt),
                                     c(xg[:], mm_dt), start=True, stop=True)
                    nc.tensor.matmul(h1t_ps2[:], c(w1a[:, 128:256], mm_dt),
                                     c(xg[:], mm_dt), start=True, stop=True)
                    nc.tensor.matmul(h1t_ps3[0:nrem, :],
                                     c(w1a[:, 256:w1cols], mm_dt),
                                     c(xg[:], mm_dt), start=True, stop=True)
                    h1s = pa.tile([128, 128], f32, tag="h1sa")
                    h1s2 = pa.tile([128, 128], f32, tag="h1sb")
                    h1s3 = pa.tile([64, 128], f32, tag="h1sc")
                    nc.any.tensor_copy(h1s[:], h1t_ps[:])
                    nc.any.tensor_copy(h1s2[:], h1t_ps2[:])
                    nc.any.tensor_copy(h1s3[0:nrem, :], h1t_ps3[0:nrem, :])
                    stg = pa.tile([128, T1W], f32, tag="stg")
                    tp = pap.tile([128, 128], f32, tag="tp")
                    nc.tensor.transpose(tp[:], h1s[:], i128[:])
                    nc.any.tensor_copy(stg[:, 0:128], tp[:])
                    tp2 = pap.tile([128, 128], f32, tag="tp")
                    nc.tensor.transpose(tp2[:], h1s2[:], i128[:])
                    nc.any.tensor_copy(stg[:, 128:256], tp2[:])
                    tp3 = pap.tile([128, 128], f32, tag="tp")
                    nc.tensor.transpose(tp3[0:128, 0:nrem], h1s3[0:nrem, :],
                                        i128[0:nrem, 0:nrem])
                    nc.any.tensor_copy(stg[:, 256:256 + nrem], tp3[:, 0:nrem])
                    # ones columns at h*(hid+1)+hid within [0:V1]
                    on = stg[:, 0:V1].rearrange("p (a b) -> p a b", b=HID + 1)
                    nc.vector.memset(on[:, :, HID], 1.0)
                    # stash al_d for this tile [128, H]
                    nc.any.tensor_copy(alds[:, t * H:(t + 1) * H],
                                       stg[:, V1 + H:V1 + 2 * H])
                    nc.sync.dma_start(
                        tbl1_shard[t * 128:(t + 1) * 128, :], stg[:])

            nc.gpsimd.collective_compute(
                "AllGather", mybir.AluOpType.bypass, replica_groups=rg,
                ins=[tbl1_shard[:]], outs=[tbl1[:]])

            # ------------------------------------------------------------
            # Phase B: layer-1 edge processing + table2 build, per tile
            # ------------------------------------------------------------
            def edge_phase(lay, tblw, tbl_full, vw, gtag, evict_fn):
                """lay: 1 or 2. vw: value width (V1 | V2). evict_fn(t, seg_ps, pools)."""
                nheads = H if lay == 1 else 1
                with tc.tile_pool(name=f"eb{lay}", bufs=2) as eb, \
                     tc.tile_pool(name=f"oh{lay}", bufs=CPT + 2) as ohp, \
                     tc.tile_pool(name=f"oht{lay}", bufs=3) as ohtp, \
                     tc.tile_pool(name=f"sc{lay}", bufs=3) as scp, \
                     tc.tile_pool(name=f"ev{lay}", bufs=2) as ev, \
                     tc.tile_pool(name=f"ps{lay}", bufs=1, space="PSUM") as ps:
                    for t in range(NT):
                        gbuf = eb.tile([128, CPT * tblw], f32, tag="gbuf")
                        if t < 2:
                            nc.vector.memset(gbuf[:], 0.0)
                        g3 = gbuf[:].rearrange("p (c w) -> p c w", w=tblw)
                        lo_i = idxlo[:, t * LOCH * 8:(t + 1) * LOCH * 8]
                        hi_i = idxhi[:, t * HICH * 8:(t + 1) * HICH * 8]
                        nc.gpsimd.dma_gather(
                            g3[:, 0:LOCH, :], tbl_full[0:cfg.split, :],
                            lo_i, LOCH * 128, LOCH * 128, tblw,
                            single_packet=False)
                        nc.gpsimd.dma_gather(
                            g3[:, LOCH:CPT, :], tbl_full[cfg.split:cfg.slots, :],
                            hi_i, HICH * 128, HICH * 128, tblw,
                            single_packet=False)

                        # one-hots + transposed one-hots
                        ohs = []
                        epre_ps = ps.tile([128, CPT * nheads], f32, tag="epre", bufs=1)
                        ep3 = epre_ps[:].rearrange("p (c h) -> p c h", h=nheads)
                        # batched al_s copy into psum (identity matmul)
                        als_view = g3[:, :, vw:vw + nheads]
                        nc.tensor.matmul(ep3, c(i128[:], f32), c(als_view, f32),
                                         start=True, stop=False, skip_group_check=True)
                        for cc in range(CPT):
                            oh = ohp.tile([128, 128], f32, tag="oh")
                            nc.vector.tensor_scalar(
                                oh[:], iota[:],
                                dstloc[:, t * CPT + cc:t * CPT + cc + 1], None,
                                mybir.AluOpType.is_equal)
                            ohs.append(oh)
                            ohT_ps = ps.tile([128, 128], f32, tag="ohT", bufs=2)
                            nc.tensor.transpose(ohT_ps[:], oh[:], i128[:])
                            ohT = ohtp.tile([128, 128], f32, tag="ohT_sb")
                            nc.any.tensor_copy(ohT[:], ohT_ps[:])
                            ald_t = (alds[:, t * H:(t + 1) * H] if lay == 1
                                     else ald2[:, t:t + 1])
                            nc.tensor.matmul(
                                ep3[:, cc, :], c(ohT[:], f32), c(ald_t, f32),
                                start=False, stop=(cc == CPT - 1),
                                skip_group_check=True)
                        # w = exp(clamp(lrelu(epre))) * mask
                        nh = CPT * nheads
                        elr = ev.tile([128, nh], f32, tag="elr")
                        nc.vector.tensor_scalar(
                            elr[:], epre_ps[:], NEG_SLOPE, None,
                            mybir.AluOpType.mult)
                        nc.vector.tensor_tensor(elr[:], elr[:], epre_ps[:],
                                                mybir.AluOpType.max)
                        nc.vector.tensor_scalar(elr[:], elr[:], CLAMP, None,
                                                mybir.AluOpType.min)
                        wexp = ev.tile([128, nh], f32, tag="wexp")
                        nc.scalar.activation(wexp[:], elr[:],
                                             mybir.ActivationFunctionType.Exp)
                        mask = ev.tile([128, CPT], f32, tag="mask")
                        nc.vector.tensor_scalar(
                            mask[:], dstloc[:, t * CPT:(t + 1) * CPT], 0.0, None,
                            mybir.AluOpType.is_ge)
                        w3 = wexp[:].rearrange("p (c h) -> p c h", h=nheads)
                        m3 = mask[:].unsqueeze(2).broadcast_to([128, CPT, nheads])
                        nc.vector.tensor_tensor(w3, w3, m3, mybir.AluOpType.mult)

                        # scale + segment-sum
                        seg_ps = ps.tile([128, vw], f32, tag="seg", bufs=2)
                        for cc in range(CPT):
                            scl = scp.tile([128, vw], f32, tag="scl")
                            s3 = scl[:].rearrange("p (h u) -> p h u", h=nheads)
                            gv = g3[:, cc, 0:vw].rearrange("p (h u) -> p h u",
                                                           h=nheads)
                            wv = w3[:, cc, :].unsqueeze(2).broadcast_to(
                                [128, nheads, vw // nheads])
                            nc.vector.tensor_tensor(s3, gv, wv,
                                                    mybir.AluOpType.mult)
                            nc.tensor.matmul(seg_ps[:], c(ohs[cc][:], seg_dt),
                                             c(scl[:], seg_dt),
                                             start=(cc == 0), stop=(cc == CPT - 1))
                        evict_fn(t, seg_ps, (eb, ev, ps))

            # ---- layer-1 eviction: ELU+1 -> table2 rows ----
            def evict1(t, seg_ps, pools):
                eb, ev, ps = pools
                sg3 = seg_ps[:].rearrange("p (h u) -> p h u", u=HID + 1)
                den = sg3[:, :, HID]                       # [128, H]
                denf = ev.tile([128, H], f32, tag="denf")
                nc.vector.tensor_scalar(denf[:], den, 0.0, None,
                                        mybir.AluOpType.is_equal)
                nc.vector.tensor_tensor(denf[:], denf[:], den,
                                        mybir.AluOpType.add)
                rec = ev.tile([128, H], f32, tag="rec")
                nc.vector.reciprocal(rec[:], denf[:])
                pe = ev.tile([128, K2], f32, tag="pelu")
                p3 = pe[:].rearrange("p (h u) -> p h u", h=H)
                r3 = rec[:].unsqueeze(2).broadcast_to([128, H, HID])
                nc.vector.tensor_tensor(p3, sg3[:, :, 0:HID], r3,
                                        mybir.AluOpType.mult)
                # pre_elu = max(v,0) + exp(min(v,0))   (== elu(v) + 1)
                mn = ev.tile([128, K2], f32, tag="mn")
                nc.vector.tensor_scalar(mn[:], pe[:], 0.0, None,
                                        mybir.AluOpType.min)
                nc.scalar.activation(mn[:], mn[:],
                                     mybir.ActivationFunctionType.Exp)
                nc.vector.tensor_scalar(pe[:], pe[:], 0.0, None,
                                        mybir.AluOpType.max)
                nc.vector.tensor_tensor(pe[:], pe[:], mn[:],
                                        mybir.AluOpType.add)
                # table2 rows: h2' = pre_elu @ W2aug + caug
                h2t_ps = ps.tile([V2 + 1, 128], f32, tag="h2t", bufs=1)
                for j in range(n_k2):
                    peT_ps = ps.tile([128, 128], f32, tag="peT", bufs=1)
                    nc.tensor.transpose(peT_ps[:], pe[:, j * 128:(j + 1) * 128],
                                        i128[:])
                    peT = ev.tile([128, 128], f32, tag="peT_sb")
                    nc.any.tensor_copy(peT[:], peT_ps[:])
                    nc.tensor.matmul(h2t_ps[:],
                                     c(w2a[:, j * (V2 + 1):(j + 1) * (V2 + 1)], f32),
                                     c(peT[:], f32),
                                     start=(j == 0), stop=False,
                                     skip_group_check=True)
                nc.tensor.matmul(h2t_ps[:], c(ca[:], f32), c(ones_row[:], f32),
                                 start=False, stop=True, skip_group_check=True)
                h2t = ev.tile([V2 + 1, 128], f32, tag="h2t_sb")
                nc.any.tensor_copy(h2t[0:V2 + 1, :], h2t_ps[0:V2 + 1, :])
                h2_ps = ps.tile([128, V2 + 1], f32, tag="h2o", bufs=1)
                nc.tensor.transpose(h2_ps[:, 0:V2 + 1], h2t[0:V2 + 1, :],
                                    i128[0:V2 + 1, 0:V2 + 1])
                stg2 = eb.tile([128, T2W], f32, tag="stg2")
                nc.any.tensor_copy(stg2[:, 0:OUT], h2_ps[:, 0:OUT])
                nc.vector.memset(stg2[:, OUT:OUT + 1], 1.0)
                nc.any.tensor_copy(stg2[:, OUT + 1:OUT + 3],
                                   h2_ps[:, OUT:OUT + 2])
                nc.any.tensor_copy(ald2[:, t:t + 1], h2_ps[:, OUT + 1:OUT + 2])
                nc.sync.dma_start(tbl2_shard[t * 128:(t + 1) * 128, :], stg2[:])

            edge_phase(1, T1W, tbl1, V1, "g1", evict1)

            nc.gpsimd.collective_compute(
                "AllGather", mybir.AluOpType.bypass, replica_groups=rg,
                ins=[tbl2_shard[:]], outs=[tbl2[:]])

            # ---- layer-2 eviction: output rows ----
            def evict2(t, seg_ps, pools):
                eb, ev, ps = pools
                den = seg_ps[:, OUT:OUT + 1]
                denf = ev.tile([128, 1], f32, tag="denf2")
                nc.vector.tensor_scalar(denf[:], den, 0.0, None,
                                        mybir.AluOpType.is_equal)
                nc.vector.tensor_tensor(denf[:], denf[:], den,
                                        mybir.AluOpType.add)
                rec = ev.tile([128, 1], f32, tag="rec2")
                nc.vector.reciprocal(rec[:], denf[:])
                ot = ev.tile([128, OUT], f32, tag="ot")
                nc.vector.tensor_scalar(ot[:], seg_ps[:, 0:OUT], rec[:], None,
                                        mybir.AluOpType.mult)
                nc.sync.dma_start(out_d[t * 128:(t + 1) * 128, :], ot[:])

            edge_phase(2, T2W, tbl2, V2, "g2", evict2)

    nc.compile()
    return nc


# ---------------------------------------------------------------------------
# Entry point
# ---------------------------------------------------------------------------

_CACHE = {}


def _get_program(cfg):
    key = tuple(sorted(cfg.__dict__.items()))
    if key not in _CACHE:
        _CACHE[key] = build_program(cfg)
    return _CACHE[key]


def run(cfg, inputs, trace=False, **kw):
    in_maps, perm = prep_host(cfg, **inputs)
    nc = _get_program(cfg)
    res = run_bass_kernel_spmd(nc, in_maps, list(range(cfg.ncores)),
                               trace=trace, **kw)
    out_full = np.concatenate([res.results[c]["out_shard"]
                               for c in range(cfg.ncores)], axis=0)
    return out_full[perm[:cfg.n]].astype(np.float32), res


def kernel(x, edge_index, W1, a_src1, a_dst1, b1, W2, a_src2, a_dst2, b2):
    out, _ = run(FULL, dict(x=x, edge_index=edge_index, W1=np.asarray(W1, np.float32),
                            a_src1=np.asarray(a_src1, np.float32),
                            a_dst1=np.asarray(a_dst1, np.float32),
                            b1=np.asarray(b1, np.float32),
                            W2=np.asarray(W2, np.float32),
                            a_src2=np.asarray(a_src2, np.float32),
                            a_dst2=np.asarray(a_dst2, np.float32),
                            b2=np.asarray(b2, np.float32)))
    return out

